# revision 4
# baseline (speedup 1.0000x reference)
"""Trainium2 Bass kernel for nn_ContinuousLocationMap (histogram binning scatter).

Reference semantics (per batch image b):
    idx = int32((batch - 0.0) / 0.0390625 + 0.5)            # [B, L, 2], trunc
    out[b, ix, iy, 0:2] = 1.0                               # corr channels
    out[b, ix, iy, 2:4] = (x, y)                            # raw location
    (duplicate bins within an image: the LAST point in l-order wins)

Full shapes: batch [256, 128, 2] f32 -> out [256, 256, 256, 4] f32.

Sharding: pure data-parallel over batch across 8 NeuronCores; each core
produces its own [32, 256, 256, 4] slice with no cross-core communication.

Per-core kernel:
  1. zero-fill the 32 MB output table with a single DMA whose access
     pattern is [128 part, 512 rep, 128 f32] rows - each 512 B row is a
     full-rate descriptor, so the fill streams at bus rate.
  2. DMA the [32, 128, 2] batch shard straight into the (x, y) channels
     of the payload tile; memset 1.0 into the corr channels.
  3. bin on DVE: one fused divide+0.5 with truncating i32 store (matches
     the reference's f32 divide + cast exactly), then row = ix*256 + iy
     + b*65536 via scalar_tensor_tensor + a per-partition iota offset.
  4. one indirect DMA scatters all 4096 rows of [1, 1, x, y] (16 B each)
     into the zeroed table. Descriptors are emitted b-major with l
     ascending, so for duplicate bins the last point in l-order wins,
     matching the reference scatter order.
"""

import numpy as np

from concourse import bass, bacc, mybir
from concourse import tile
from concourse import bass_utils

F32 = mybir.dt.float32
I32 = mybir.dt.int32

N_CORES = 8
B_FULL = 256
B = B_FULL // N_CORES  # 32 images per core
L = 128                # points per image
X = Y = 256            # bins
C = 4                  # output channels
ROWS = B * X * Y       # 2097152 table rows per core
DELTA = 0.0390625      # (10.0 - 0.0) / 256, exact in f32


def _build_nc() -> bass.Bass:
    nc = bacc.Bacc("TRN2", target_bir_lowering=False)

    batch_d = nc.declare_dram_parameter("batch", [B, L, 2], F32, isOutput=False)
    table_d = nc.declare_dram_parameter("out", [ROWS, C], F32, isOutput=True)

    with tile.TileContext(nc) as tc:
        with (
            tc.tile_pool(name="const", bufs=1) as cpool,
            tc.tile_pool(name="work", bufs=1) as wpool,
        ):
            # ---- per-partition row base b*65536 (no host constant needed) ----
            boff = cpool.tile([B, 1], F32)
            nc.gpsimd.iota(
                boff[:], pattern=[[0, 1]], base=0,
                channel_multiplier=X * Y,
                allow_small_or_imprecise_dtypes=True,
            )

            # ---- payload tile [B, L, 4] = rows of [1, 1, x, y] ----
            pay = wpool.tile([B, L, C], F32)
            nc.gpsimd.memset(pay[:, :, 0:2], 1.0)
            nc.scalar.dma_start(out=pay[:, :, 2:4], in_=batch_d[:])

            # ---- 1. zero-fill the 32 MB table in one streaming DMA ----
            # 1 KB rows; the lowered AP's merged outer dim (128*256) must
            # stay <= 65535 to fit the ISA's 16-bit num_elem field.
            z = cpool.tile([128, 256], F32)
            nc.gpsimd.memset(z[:], 0.0)
            tview = table_d[:].rearrange("(p r f) c -> p r (f c)", p=128, r=256)
            nc.sync.dma_start(
                out=tview,
                in_=z[:].unsqueeze(1).to_broadcast([128, 256, 256]),
            )

            # ---- 2. binning: i = trunc(x/DELTA + 0.5) (store to i32) ----
            q = wpool.tile([B, L, 2], I32)
            nc.vector.tensor_scalar(
                out=q[:], in0=pay[:, :, 2:4],
                scalar1=float(DELTA), scalar2=0.5,
                op0=mybir.AluOpType.divide, op1=mybir.AluOpType.add,
            )
            # row = (ix*256 + iy) + b*65536
            row = wpool.tile([B, L], I32)
            nc.vector.scalar_tensor_tensor(
                out=row[:], in0=q[:, :, 0], scalar=256.0, in1=q[:, :, 1],
                op0=mybir.AluOpType.mult, op1=mybir.AluOpType.add,
            )
            nc.vector.tensor_scalar(
                out=row[:], in0=row[:],
                scalar1=boff[:, 0:1], scalar2=None,
                op0=mybir.AluOpType.add,
            )

            # ---- 3. scatter all 4096 points in one indirect DMA.
            # Descriptor order is b-major, l ascending -> last-writer-wins
            # for duplicate bins, same as the reference scatter.
            nc.gpsimd.indirect_dma_start(
                out=table_d[:],
                out_offset=bass.IndirectOffsetOnAxis(ap=row[:], axis=0),
                in_=pay[:],
                in_offset=None,
                bounds_check=ROWS - 1,
                oob_is_err=False,
            )

    nc.compile()
    return nc


_NC_CACHE = None


def _get_nc() -> bass.Bass:
    global _NC_CACHE
    if _NC_CACHE is None:
        _NC_CACHE = _build_nc()
    return _NC_CACHE


def _host_constants() -> dict[str, np.ndarray]:
    return {}


def run_sharded(batch: np.ndarray, **spmd_kwargs):
    """Shard batch over the 8 cores, run the Bass kernel, return raw results."""
    batch = np.ascontiguousarray(np.asarray(batch, dtype=np.float32))
    assert batch.shape == (B_FULL, L, 2), batch.shape
    shards = np.split(batch, N_CORES, axis=0)
    in_maps = [{"batch": np.ascontiguousarray(s)} for s in shards]
    nc = _get_nc()
    return bass_utils.run_bass_kernel_spmd(
        nc, in_maps, core_ids=list(range(N_CORES)), **spmd_kwargs
    )


def kernel(batch: np.ndarray) -> np.ndarray:
    res = run_sharded(batch)
    parts = [r["out"].reshape(B, X, Y, C) for r in res.results]
    return np.concatenate(parts, axis=0)
